# revision 41
# baseline (speedup 1.0000x reference)
"""Multi-head attention on 8 NeuronCores (Trainium2, Bass/Tile).

Problem: B=2, S=2048, E=1024, H=16, D=64 MHA with int mask, fp32.

Sharding: core c = 4*b + g handles batch b, head group g (4 heads = a
256-wide slice of E).  Q/K/V projections, scores, softmax and attention
are head-parallel; Wo is row-sharded so each core emits a partial [S, E]
output projection; the host sums the 4 partials per batch and adds bo.

Device schedule per core (software-pipelined emission):
  The ACT-paced stream is scores->exp over all (qc, h, ks); everything
  else (projection column blocks, v-projection s-tiles, ctx accumulation
  groups, transposes + out projection) is emitted as small PE "filler"
  units interleaved between score tiles so no engine sits behind a phase
  barrier.

  qhT, khT : [j, S] fp16 pair-major [128, pair, S]; PE from fp16 inputs,
             built in 512-column blocks so attention starts early.
  vh       : [128 ks, st, head, 65] fp16 with a ones column (denominator).
  scores   : [ks, q] fp16 matmuls (K=64, head quadrants) into PSUM;
             ACT exp (scale=1/8) -> P staged [128, 16, 1024] fp16 in SBUF;
             DVE mask multiply (fp16 2x mode).
  ctx      : [q, d] orientation: per q-tile, 16 accumulation matmuls of
             65 cols (K=128) -> PSUM [128q, 65]; per-partition reciprocal
             of the ones column normalizes on DVE (tensor_scalar).
  ctxT     : PE transpose (identity matmul) back to [j, q] fp16, then the
             partial out projection; Pool engine drains out PSUM to SBUF.
"""

import os
import sys
from collections import deque

sys.path.insert(0, "/opt/trn_rl_repo")

import numpy as np

import concourse.mybir as mybir
import concourse.tile as tile
from concourse import bacc
from concourse import bass_utils
from concourse.masks import make_identity

B, S, E, H = 2, 2048, 1024, 16
D = E // H              # 64
G = 4                   # head groups (cores per batch)
HL = H // G             # 4 local heads per core
J = HL * D              # 256 local j width
P = 128
KT = E // P             # 8 k-tiles for projections
ST = S // P             # 16 ks-tiles
NQ = 1024               # q-chunk width for attention
QC = S // NQ            # 2 q chunks
QT = NQ // P            # 8 q-tiles per chunk
NB = S // 512           # 4 projection column blocks per tensor

F32 = mybir.dt.float32
F16 = mybir.dt.float16

# Exposed for test.py / bench.py.
LAST_RESULTS = None
LAST_NC = None


def _f16(x: np.ndarray) -> np.ndarray:
    return np.ascontiguousarray(x, dtype=np.float32).astype(np.float16)


def _build_program(use_bias_qk: bool, use_bias_v: bool):
    nc = bacc.Bacc("TRN2", target_bir_lowering=False, debug=False, num_devices=8)

    xqT = nc.dram_tensor("xqT", [E, S], F16, kind="ExternalInput")
    xkT = nc.dram_tensor("xkT", [E, S], F16, kind="ExternalInput")
    xvT = nc.dram_tensor("xvT", [E, S], F16, kind="ExternalInput")
    maskT = nc.dram_tensor("maskT", [S, S], F16, kind="ExternalInput")
    wqT = nc.dram_tensor("wqT", [E, J], F16, kind="ExternalInput")
    wkT = nc.dram_tensor("wkT", [E, J], F16, kind="ExternalInput")
    wvT = nc.dram_tensor("wvT", [E, J], F16, kind="ExternalInput")
    woT = nc.dram_tensor("woT", [J, E], F16, kind="ExternalInput")
    bq = nc.dram_tensor("bq", [J], F32, kind="ExternalInput")
    bk = nc.dram_tensor("bk", [J], F32, kind="ExternalInput")
    bv = nc.dram_tensor("bv", [J], F32, kind="ExternalInput")
    out = nc.dram_tensor("out", [S, E], F16, kind="ExternalOutput")

    Copy = mybir.ActivationFunctionType.Copy
    Identity = mybir.ActivationFunctionType.Identity
    Exp = mybir.ActivationFunctionType.Exp

    with tile.TileContext(nc) as tc:
        with (
            tc.tile_pool(name="consts", bufs=1) as consts,
            tc.tile_pool(name="persist", bufs=1) as persist,
            tc.tile_pool(name="xs", bufs=6) as xs,
            tc.tile_pool(name="xv", bufs=2) as xvpool,
            tc.tile_pool(name="maskp", bufs=4) as maskp,
            tc.tile_pool(name="pstg", bufs=8) as pstg,
            tc.tile_pool(name="small", bufs=8) as small,
            tc.tile_pool(name="osb", bufs=3) as osb,
            tc.tile_pool(name="scps", bufs=2, space="PSUM") as scps,
            tc.tile_pool(name="wkps", bufs=4, space="PSUM") as wkps,
        ):
            # ---- weights / constants ----
            wq_sb = consts.tile([P, KT, J], F16, tag="wq")
            wk_sb = consts.tile([P, KT, J], F16, tag="wk")
            wv_sb = consts.tile([P, KT, J], F16, tag="wv")
            wo_sb = consts.tile([P, 2, E], F16, tag="wo")
            ident = consts.tile([P, P], F16, tag="ident")
            make_identity(nc, ident[:])

            if use_bias_qk:
                bq_sb = consts.tile([P, 2], F32, tag="bq")
                bk_sb = consts.tile([P, 2], F32, tag="bk")
                nc.sync.dma_start(bq_sb[:], bq.rearrange("(pr p) -> p pr", p=P))
                nc.sync.dma_start(bk_sb[:], bk.rearrange("(pr p) -> p pr", p=P))
            if use_bias_v:
                bv_row = consts.tile([1, J], F32, tag="bvr")
                nc.sync.dma_start(bv_row[:], bv.rearrange("j -> 1 j"))
                bv_bc = consts.tile([P, J], F32, tag="bvb")
                nc.gpsimd.partition_broadcast(bv_bc[:], bv_row[:])

            # ---- persistent activations ----
            qhT = persist.tile([P, 2, S], F16, tag="qhT")
            khT = persist.tile([P, 2, S], F16, tag="khT")
            vh = persist.tile([P, ST, HL, 65], F16, tag="vh")
            ctxn = persist.tile([P, QT, HL, D], F16, tag="ctxn")
            ctxT = persist.tile([P, 2, NQ], F16, tag="ctxT")

            nc.gpsimd.memset(vh[:, :, :, 64:65], 1.0)

            # ---- projection units (column-blocked, pair-split) ----
            # Each unit computes ONE 512-column block of ONE head-pair of a
            # projection (8 matmuls + 1 copy) and re-DMAs its own x chunk, so
            # the pre-stream critical path holds only the pair-0 blocks the
            # first two heads need.
            vchunks = {}
            xv_next = [0]

            def emit_xv_dma():
                c = xv_next[0]
                if c < 8:
                    xv_next[0] += 1
                    t = xvpool.tile([P, KT, 256], F16, tag="vch",
                                    name=f"vch{c}")
                    nc.sync.dma_start(
                        t[:],
                        xvT[:, c * 256:(c + 1) * 256].rearrange(
                            "(kt p) q -> p kt q", p=P))
                    vchunks[c] = t

            chunk_store = {}

            def chunk_dma(kind, x_dram, blk):
                t = xs.tile([P, KT, 512], F16, tag="xc",
                            name=f"xc_{kind}{blk}")
                nc.sync.dma_start(
                    t[:], x_dram[:, blk * 512:(blk + 1) * 512].rearrange(
                        "(kt p) q -> p kt q", p=P))
                chunk_store[(kind, blk)] = t

            def mk_proj_pair(kind, w_sb, x_dram, outT, b_sb, blk, pair,
                             act_copy=False):
                def dma():
                    chunk_dma(kind, x_dram, blk)

                def mm():
                    chunks = [chunk_store[(kind, blk)]]
                    acc = wkps.tile([P, 512], F32, tag="wk",
                                    name=f"pj_{kind}{blk}_{pair}")
                    for kt in range(KT):
                        nc.tensor.matmul(
                            acc[:],
                            w_sb[:, kt, pair * P:(pair + 1) * P],
                            chunks[0][:, kt, :],
                            start=(kt == 0), stop=(kt == KT - 1),
                        )
                    dst = outT[:, pair, blk * 512:(blk + 1) * 512]
                    if use_bias_qk:
                        nc.scalar.activation(dst, acc[:], Identity,
                                             bias=b_sb[:, pair:pair + 1])
                    elif act_copy:
                        nc.scalar.activation(dst, acc[:], Copy)
                    else:
                        nc.vector.tensor_copy(dst, acc[:])

                return dma, mm

            def mk_v_unit(st_):
                def mm():
                    vacc = wkps.tile([P, J], F32, tag="wk", name=f"vacc{st_}")
                    vch = vchunks[st_ // 2]
                    sl = slice((st_ % 2) * P, (st_ % 2 + 1) * P)
                    for kt in range(KT):
                        nc.tensor.matmul(
                            vacc[:],
                            vch[:, kt, sl],
                            wv_sb[:, kt, :],
                            start=(kt == 0), stop=(kt == KT - 1),
                        )
                    src3 = vacc[:].rearrange("p (h d) -> p h d", h=HL)
                    dst = vh[:, st_, :, 0:64]
                    if use_bias_v:
                        nc.vector.tensor_add(
                            dst, src3, bv_bc[:].rearrange("p (h d) -> p h d", h=HL))
                    else:
                        nc.vector.tensor_copy(dst, src3)
                return mm

            bqs = bq_sb if use_bias_qk else None
            bks = bk_sb if use_bias_qk else None

            def kh_unit(blk, pair, act_copy=False):
                return mk_proj_pair("k", wk_sb, xkT, khT, bks, blk, pair,
                                    act_copy)

            def qh_unit(blk, pair, act_copy=False):
                return mk_proj_pair("q", wq_sb, xqT, qhT, bqs, blk, pair,
                                    act_copy)

            # ---- filler machinery ----
            fillers = deque()

            def pop_filler():
                if fillers:
                    fillers.popleft()[1]()

            def mk_stage2(qc, h, qt, p_ab):
                def unit():
                    cps = wkps.tile([P, 512], F32, tag="wk",
                                    name=f"c{qc}_{h}_{qt}")
                    for ks in range(ST):
                        nc.tensor.matmul(
                            cps[:, 0:65],
                            p_ab[ks // 4][:, ks % 4, qt * P:(qt + 1) * P],
                            vh[:, ks, h, :],
                            start=(ks == 0), stop=(ks == ST - 1),
                        )
                    rcp = small.tile([P, 1], F32, tag="rcp",
                                     name=f"r{qc}_{h}_{qt}")
                    nc.vector.reciprocal(rcp[:], cps[:, 64:65])
                    if qc == 1 and h == HL - 1:
                        nc.scalar.mul(ctxn[:, qt, h, :], cps[:, 0:64], rcp[:])
                    else:
                        nc.vector.tensor_scalar_mul(
                            ctxn[:, qt, h, :], cps[:, 0:64], rcp[:])
                    if h == HL - 1:
                        fillers.append(("c1", mk_C1(qc, qt)))
                return unit

            def mk_C1(qc, qt):
                def unit():
                    tp = wkps.tile([P, 1024], F16, tag="wk",
                                   name=f"tp{qc}_{qt}")
                    for hp2 in range(2):
                        nc.tensor.matmul(
                            tp[:, hp2 * P:(hp2 + 1) * P],
                            ctxn[:, qt, 2 * hp2:2 * hp2 + 2, :],
                            ident[:],
                            is_transpose=True,
                            start=(hp2 == 0), stop=(hp2 == 1),
                        )
                    nc.vector.tensor_copy(
                        ctxT[:, :, qt * P:(qt + 1) * P],
                        tp[:, 0:256].rearrange("p (two q) -> p two q", two=2),
                    )
                    fillers.append(("c2", mk_C2(qc, qt)))
                return unit

            def mk_C2(qc, qt):
                def unit():
                    o_sb = osb.tile([P, E], F16, tag="o", name=f"os{qc}_{qt}")
                    for ec in range(2):
                        ops = wkps.tile([P, 512], F32, tag="wk",
                                        name=f"op{qc}_{qt}_{ec}")
                        for hp2 in range(2):
                            nc.tensor.matmul(
                                ops[:],
                                ctxT[:, hp2, qt * P:(qt + 1) * P],
                                wo_sb[:, hp2, ec * 512:(ec + 1) * 512],
                                start=(hp2 == 0), stop=(hp2 == 1),
                            )
                        if qc == 1:
                            nc.scalar.activation(
                                o_sb[:, ec * 512:(ec + 1) * 512], ops[:], Copy)
                        else:
                            nc.vector.tensor_copy(
                                o_sb[:, ec * 512:(ec + 1) * 512], ops[:])
                    nc.sync.dma_start(
                        out[qc * NQ + qt * P:qc * NQ + (qt + 1) * P, :], o_sb[:])
                return unit

            # ---- prologue: pair-0 of kh block 0 and qh blocks 0-1 is all
            # the score stream needs to start; everything else is deferred.
            kh00 = kh_unit(0, 0)
            qh00 = qh_unit(0, 0, act_copy=True)
            qh10 = qh_unit(1, 0)
            nc.sync.dma_start(wk_sb[:], wkT.rearrange("(kt p) j -> p kt j", p=P))
            kh00[0]()
            nc.sync.dma_start(wq_sb[:], wqT.rearrange("(kt p) j -> p kt j", p=P))
            qh00[0]()
            qh10[0]()
            kh00[1]()
            qh00[1]()
            qh10[1]()

            mtiles = {}

            def mask_dma(qc, c):
                mch = maskp.tile([P, 4, NQ], F16, tag="mch",
                                 name=f"mch{qc}_{c}")
                nc.sync.dma_start(
                    mch[:],
                    maskT[c * 4 * P:(c + 1) * 4 * P,
                          qc * NQ:(qc + 1) * NQ].rearrange(
                        "(kt p) q -> p kt q", p=P),
                )
                mtiles[(qc, c)] = mch

            # proj chain: pair-0 kh blocks first (the stream consumes them
            # within the first head), then the deferred pair-1 blocks, then
            # the v s-tiles.  Each filler prefetches the next unit's chunk.
            chain = ([kh_unit(b, 0) for b in (1, 2, 3)]
                     + [kh_unit(b, 1) for b in range(4)]
                     + [qh_unit(b, 1) for b in (0, 1)])
            chunk_dma("k", xkT, 1)
            mask_dma(0, 0)
            chunk_dma("k", xkT, 2)
            mask_dma(0, 1)

            def mk_chain_filler(i):
                def unit():
                    if i == 0:
                        chunk_dma("k", xkT, 3)
                    elif i == 1:
                        nc.sync.dma_start(
                            wv_sb[:],
                            wvT.rearrange("(kt p) j -> p kt j", p=P))
                        emit_xv_dma()
                    chain[i][1]()
                return unit

            for i in range(len(chain)):
                fillers.append(("p", mk_chain_filler(i)))

            def mk_v_filler(st_):
                vu = mk_v_unit(st_)
                def unit():
                    if st_ % 2 == 0:
                        emit_xv_dma()
                    vu()
                return unit

            for st_ in range(ST):
                fillers.append(("p", mk_v_filler(st_)))

            qh_tail = [qh_unit(2, 0), qh_unit(3, 0),
                       qh_unit(2, 1), qh_unit(3, 1)]

            def mk_qh_tail_filler(i):
                def unit():
                    if i == 0:
                        chunk_dma("q", xqT, 3)
                    elif i == 1:
                        nc.sync.dma_start(
                            wo_sb[:], woT.rearrange("(hp p) e -> p hp e", p=P))
                    qh_tail[i][1]()
                return unit

            # ---- main ACT-paced loop ----
            for qc in range(QC):
                for h in range(HL):
                    hp, hh = divmod(h, 2)
                    p_ab = tuple(
                        pstg.tile([P, 4, NQ], F16, tag="pt",
                                  name=f"pt{qc}_{h}_{i}")
                        for i in range(4))
                    for pr in range(ST // 2):
                        if h == 0 and (qc, pr // 2) not in mtiles:
                            mask_dma(qc, pr // 2)
                        mcur = mtiles[(qc, pr // 2)]
                        for k2 in range(2):
                            ks = pr * 2 + k2
                            st_ = scps.tile([P, NQ], F32, tag="st")
                            for n2 in range(2):
                                nc.tensor.matmul(
                                    st_[:, n2 * 512:(n2 + 1) * 512],
                                    khT[hh * 64:(hh + 1) * 64, hp,
                                        ks * P:(ks + 1) * P],
                                    qhT[hh * 64:(hh + 1) * 64, hp,
                                        qc * NQ + n2 * 512:
                                        qc * NQ + (n2 + 1) * 512],
                                    start=True, stop=True,
                                )
                            ph = p_ab[ks // 4][:, ks % 4, :]
                            nc.scalar.activation(ph, st_[:], Exp, scale=0.125)
                            eng = nc.gpsimd if ks % 8 == 3 else nc.vector
                            eng.tensor_mul(ph, ph, mcur[:, ks % 4, :])
                            pop_filler()
                    for qt in range(QT):
                        fillers.append(("s2", mk_stage2(qc, h, qt, p_ab)))
                    if qc == 0 and h == 0:
                        chunk_dma("q", xqT, 2)
                        for i in range(len(qh_tail)):
                            fillers.append(("p", mk_qh_tail_filler(i)))

            prio = {"p": 0, "s2": 1, "c1": 2, "c2": 3}
            while fillers:
                batch = sorted(fillers, key=lambda kf: prio[kf[0]])
                fillers.clear()
                for _, fn in batch:
                    fn()

    nc.compile()
    return nc


def kernel(q, k, v, mask, Wq, bq, Wk, bk, Wv, bv, Wo, bo):
    global LAST_RESULTS, LAST_NC
    q = np.asarray(q, np.float32)
    k = np.asarray(k, np.float32)
    v = np.asarray(v, np.float32)
    mask = np.asarray(mask)
    Wq = np.asarray(Wq, np.float32)
    Wk = np.asarray(Wk, np.float32)
    Wv = np.asarray(Wv, np.float32)
    Wo = np.asarray(Wo, np.float32)
    bq = np.asarray(bq, np.float32)
    bk = np.asarray(bk, np.float32)
    bv = np.asarray(bv, np.float32)
    bo = np.asarray(bo, np.float32)

    use_bias_qk = bool(np.any(bq) or np.any(bk))
    use_bias_v = bool(np.any(bv))

    nc = _build_program(use_bias_qk, use_bias_v)
    LAST_NC = nc

    xT = {}
    for b in range(B):
        xT[("q", b)] = _f16(q[b].T)
        xT[("k", b)] = _f16(k[b].T)
        xT[("v", b)] = _f16(v[b].T)
        xT[("m", b)] = _f16(mask[b, 0].T.astype(np.float32))

    in_maps = []
    for c in range(8):
        b, g = divmod(c, G)
        js = slice(g * J, (g + 1) * J)
        in_maps.append({
            "xqT": xT[("q", b)],
            "xkT": xT[("k", b)],
            "xvT": xT[("v", b)],
            "maskT": xT[("m", b)],
            "wqT": _f16(Wq[js, :].T),
            "wkT": _f16(Wk[js, :].T),
            "wvT": _f16(Wv[js, :].T),
            "woT": _f16(Wo[:, js].T),
            "bq": np.ascontiguousarray(bq[js]),
            "bk": np.ascontiguousarray(bk[js]),
            "bv": np.ascontiguousarray(bv[js]),
        })

    os.environ["BASS_NEVER_TRACE"] = "1"
    res = bass_utils.run_bass_kernel_spmd(
        nc, in_maps, core_ids=list(range(8)), trace=False,
    )
    LAST_RESULTS = res

    full = np.zeros((B, S, E), np.float32)
    for c in range(8):
        b = c // G
        full[b] += res.results[c]["out"].astype(np.float32)
    full += bo[None, None, :]
    return full


# revision 46
# speedup vs baseline: 1.0193x; 1.0193x over previous
"""Multi-head attention on 8 NeuronCores (Trainium2, Bass/Tile).

Problem: B=2, S=2048, E=1024, H=16, D=64 MHA with int mask, fp32.

Sharding: core c = 4*b + g handles batch b, head group g (4 heads = a
256-wide slice of E).  Q/K/V projections, scores, softmax and attention
are head-parallel; Wo is row-sharded so each core emits a partial [S, E]
output projection; the host sums the 4 partials per batch and adds bo.

Device schedule per core (software-pipelined emission):
  The ACT-paced stream is scores->exp over all (qc, h, ks); everything
  else (projection column blocks, v-projection s-tiles, ctx accumulation
  groups, transposes + out projection) is emitted as small PE "filler"
  units interleaved between score tiles so no engine sits behind a phase
  barrier.

  qhT, khT : [j, S] fp16 pair-major [128, pair, S]; PE from fp16 inputs,
             built in 512-column blocks so attention starts early.
  vh       : [128 ks, st, head, 65] fp16 with a ones column (denominator).
  scores   : [ks, q] fp16 matmuls (K=64, head quadrants) into PSUM;
             ACT exp (scale=1/8) -> P staged [128, 16, 1024] fp16 in SBUF;
             DVE mask multiply (fp16 2x mode).
  ctx      : [q, d] orientation: per q-tile, 16 accumulation matmuls of
             65 cols (K=128) -> PSUM [128q, 65]; per-partition reciprocal
             of the ones column normalizes on DVE (tensor_scalar).
  ctxT     : PE transpose (identity matmul) back to [j, q] fp16, then the
             partial out projection; Pool engine drains out PSUM to SBUF.
"""

import os
import sys
from collections import deque

sys.path.insert(0, "/opt/trn_rl_repo")

import numpy as np

import concourse.mybir as mybir
import concourse.tile as tile
from concourse import bacc
from concourse import bass_utils
from concourse.masks import make_identity

B, S, E, H = 2, 2048, 1024, 16
D = E // H              # 64
G = 4                   # head groups (cores per batch)
HL = H // G             # 4 local heads per core
J = HL * D              # 256 local j width
P = 128
KT = E // P             # 8 k-tiles for projections
ST = S // P             # 16 ks-tiles
NQ = 1024               # q-chunk width for attention
QC = S // NQ            # 2 q chunks
QT = NQ // P            # 8 q-tiles per chunk
NB = S // 512           # 4 projection column blocks per tensor

F32 = mybir.dt.float32
F16 = mybir.dt.float16

# Exposed for test.py / bench.py.
LAST_RESULTS = None
LAST_NC = None


def _f16(x: np.ndarray) -> np.ndarray:
    return np.ascontiguousarray(x, dtype=np.float32).astype(np.float16)


def _build_program(use_bias_qk: bool, use_bias_v: bool):
    nc = bacc.Bacc("TRN2", target_bir_lowering=False, debug=False, num_devices=8)

    xqT = nc.dram_tensor("xqT", [E, S], F16, kind="ExternalInput")
    xkT = nc.dram_tensor("xkT", [E, S], F16, kind="ExternalInput")
    xvT = nc.dram_tensor("xvT", [E, S], F16, kind="ExternalInput")
    maskT = nc.dram_tensor("maskT", [S, S], F16, kind="ExternalInput")
    wqT = nc.dram_tensor("wqT", [E, J], F16, kind="ExternalInput")
    wkT = nc.dram_tensor("wkT", [E, J], F16, kind="ExternalInput")
    wvT = nc.dram_tensor("wvT", [E, J], F16, kind="ExternalInput")
    woT = nc.dram_tensor("woT", [J, E], F16, kind="ExternalInput")
    bq = nc.dram_tensor("bq", [J], F32, kind="ExternalInput")
    bk = nc.dram_tensor("bk", [J], F32, kind="ExternalInput")
    bv = nc.dram_tensor("bv", [J], F32, kind="ExternalInput")
    out = nc.dram_tensor("out", [S, E], F16, kind="ExternalOutput")

    Copy = mybir.ActivationFunctionType.Copy
    Identity = mybir.ActivationFunctionType.Identity
    Exp = mybir.ActivationFunctionType.Exp

    with tile.TileContext(nc) as tc:
        with (
            tc.tile_pool(name="consts", bufs=1) as consts,
            tc.tile_pool(name="persist", bufs=1) as persist,
            tc.tile_pool(name="xs", bufs=6) as xs,
            tc.tile_pool(name="xv", bufs=2) as xvpool,
            tc.tile_pool(name="maskp", bufs=4) as maskp,
            tc.tile_pool(name="pstg", bufs=8) as pstg,
            tc.tile_pool(name="small", bufs=8) as small,
            tc.tile_pool(name="osb", bufs=3) as osb,
            tc.tile_pool(name="scps", bufs=2, space="PSUM") as scps,
            tc.tile_pool(name="wkps", bufs=4, space="PSUM") as wkps,
        ):
            # ---- weights / constants ----
            wq_sb = consts.tile([P, KT, J], F16, tag="wq")
            wk_sb = consts.tile([P, KT, J], F16, tag="wk")
            wv_sb = consts.tile([P, KT, J], F16, tag="wv")
            wo_sb = consts.tile([P, 2, E], F16, tag="wo")
            ident = consts.tile([P, P], F16, tag="ident")
            make_identity(nc, ident[:])

            if use_bias_qk:
                bq_sb = consts.tile([P, 2], F32, tag="bq")
                bk_sb = consts.tile([P, 2], F32, tag="bk")
                nc.sync.dma_start(bq_sb[:], bq.rearrange("(pr p) -> p pr", p=P))
                nc.sync.dma_start(bk_sb[:], bk.rearrange("(pr p) -> p pr", p=P))
            if use_bias_v:
                bv_row = consts.tile([1, J], F32, tag="bvr")
                nc.sync.dma_start(bv_row[:], bv.rearrange("j -> 1 j"))
                bv_bc = consts.tile([P, J], F32, tag="bvb")
                nc.gpsimd.partition_broadcast(bv_bc[:], bv_row[:])

            # ---- persistent activations ----
            qhT = persist.tile([P, 2, S], F16, tag="qhT")
            khT = persist.tile([P, 2, S], F16, tag="khT")
            vh = persist.tile([P, ST, HL, 65], F16, tag="vh")
            ctxn = persist.tile([P, QT, HL, D], F16, tag="ctxn")
            ctxT = persist.tile([P, 2, NQ], F16, tag="ctxT")

            nc.gpsimd.memset(vh[:, :, :, 64:65], 1.0)

            # ---- projection units (column-blocked, pair-split) ----
            # Each unit computes ONE 512-column block of ONE head-pair of a
            # projection (8 matmuls + 1 copy) and re-DMAs its own x chunk, so
            # the pre-stream critical path holds only the pair-0 blocks the
            # first two heads need.
            vchunks = {}
            xv_next = [0]

            def emit_xv_dma():
                c = xv_next[0]
                if c < 8:
                    xv_next[0] += 1
                    t = xvpool.tile([P, KT, 256], F16, tag="vch",
                                    name=f"vch{c}")
                    nc.sync.dma_start(
                        t[:],
                        xvT[:, c * 256:(c + 1) * 256].rearrange(
                            "(kt p) q -> p kt q", p=P))
                    vchunks[c] = t

            chunk_store = {}

            def chunk_dma(kind, x_dram, blk):
                t = xs.tile([P, KT, 512], F16, tag="xc",
                            name=f"xc_{kind}{blk}")
                nc.sync.dma_start(
                    t[:], x_dram[:, blk * 512:(blk + 1) * 512].rearrange(
                        "(kt p) q -> p kt q", p=P))
                chunk_store[(kind, blk)] = t

            def mk_proj_pair(kind, w_sb, x_dram, outT, b_sb, blk, pair,
                             act_copy=False):
                def dma():
                    chunk_dma(kind, x_dram, blk)

                def mm():
                    chunks = [chunk_store[(kind, blk)]]
                    acc = wkps.tile([P, 512], F32, tag="wk",
                                    name=f"pj_{kind}{blk}_{pair}")
                    for kt in range(KT):
                        nc.tensor.matmul(
                            acc[:],
                            w_sb[:, kt, pair * P:(pair + 1) * P],
                            chunks[0][:, kt, :],
                            start=(kt == 0), stop=(kt == KT - 1),
                        )
                    dst = outT[:, pair, blk * 512:(blk + 1) * 512]
                    if use_bias_qk:
                        nc.scalar.activation(dst, acc[:], Identity,
                                             bias=b_sb[:, pair:pair + 1])
                    elif act_copy:
                        nc.scalar.activation(dst, acc[:], Copy)
                    else:
                        nc.vector.tensor_copy(dst, acc[:])

                return dma, mm

            def mk_v_unit(st_):
                def mm():
                    vacc = wkps.tile([P, J], F32, tag="wk", name=f"vacc{st_}")
                    vch = vchunks[st_ // 2]
                    sl = slice((st_ % 2) * P, (st_ % 2 + 1) * P)
                    for kt in range(KT):
                        nc.tensor.matmul(
                            vacc[:],
                            vch[:, kt, sl],
                            wv_sb[:, kt, :],
                            start=(kt == 0), stop=(kt == KT - 1),
                        )
                    src3 = vacc[:].rearrange("p (h d) -> p h d", h=HL)
                    dst = vh[:, st_, :, 0:64]
                    if use_bias_v:
                        nc.vector.tensor_add(
                            dst, src3, bv_bc[:].rearrange("p (h d) -> p h d", h=HL))
                    else:
                        nc.vector.tensor_copy(dst, src3)
                return mm

            bqs = bq_sb if use_bias_qk else None
            bks = bk_sb if use_bias_qk else None

            def kh_unit(blk, pair, act_copy=False):
                return mk_proj_pair("k", wk_sb, xkT, khT, bks, blk, pair,
                                    act_copy)

            def qh_unit(blk, pair, act_copy=False):
                return mk_proj_pair("q", wq_sb, xqT, qhT, bqs, blk, pair,
                                    act_copy)

            # ---- filler machinery ----
            fillers = deque()

            def pop_filler():
                if fillers:
                    fillers.popleft()[1]()

            def mk_stage2(qc, h, qt, p_ab):
                def unit():
                    cps = wkps.tile([P, 512], F32, tag="wk",
                                    name=f"c{qc}_{h}_{qt}")
                    for ks in range(ST):
                        nc.tensor.matmul(
                            cps[:, 0:65],
                            p_ab[ks // 4][:, ks % 4, qt * P:(qt + 1) * P],
                            vh[:, ks, h, :],
                            start=(ks == 0), stop=(ks == ST - 1),
                        )
                    rcp = small.tile([P, 1], F32, tag="rcp",
                                     name=f"r{qc}_{h}_{qt}")
                    nc.vector.reciprocal(rcp[:], cps[:, 64:65])
                    if qc == 1 and h == HL - 1:
                        nc.scalar.mul(ctxn[:, qt, h, :], cps[:, 0:64], rcp[:])
                    else:
                        nc.vector.tensor_scalar_mul(
                            ctxn[:, qt, h, :], cps[:, 0:64], rcp[:])
                    if h == HL - 1:
                        fillers.append(("c1", mk_C1(qc, qt)))
                return unit

            def mk_C1(qc, qt):
                def unit():
                    tp = wkps.tile([P, 1024], F16, tag="wk",
                                   name=f"tp{qc}_{qt}")
                    for hp2 in range(2):
                        nc.tensor.matmul(
                            tp[:, hp2 * P:(hp2 + 1) * P],
                            ctxn[:, qt, 2 * hp2:2 * hp2 + 2, :],
                            ident[:],
                            is_transpose=True,
                            start=(hp2 == 0), stop=(hp2 == 1),
                        )
                    nc.vector.tensor_copy(
                        ctxT[:, :, qt * P:(qt + 1) * P],
                        tp[:, 0:256].rearrange("p (two q) -> p two q", two=2),
                    )
                    fillers.append(("c2", mk_C2(qc, qt)))
                return unit

            def mk_C2(qc, qt):
                def unit():
                    o_sb = osb.tile([P, E], F16, tag="o", name=f"os{qc}_{qt}")
                    for ec in range(2):
                        ops = wkps.tile([P, 512], F32, tag="wk",
                                        name=f"op{qc}_{qt}_{ec}")
                        for hp2 in range(2):
                            nc.tensor.matmul(
                                ops[:],
                                ctxT[:, hp2, qt * P:(qt + 1) * P],
                                wo_sb[:, hp2, ec * 512:(ec + 1) * 512],
                                start=(hp2 == 0), stop=(hp2 == 1),
                            )
                        if qc == 1 and ec == 1:
                            nc.scalar.activation(
                                o_sb[:, ec * 512:(ec + 1) * 512], ops[:], Copy)
                        else:
                            nc.vector.tensor_copy(
                                o_sb[:, ec * 512:(ec + 1) * 512], ops[:])
                    nc.sync.dma_start(
                        out[qc * NQ + qt * P:qc * NQ + (qt + 1) * P, :], o_sb[:])
                return unit

            # ---- prologue: pair-0 of kh block 0 and qh blocks 0-1 is all
            # the score stream needs to start; everything else is deferred.
            kh00 = kh_unit(0, 0)
            qh00 = qh_unit(0, 0, act_copy=True)
            qh10 = qh_unit(1, 0)
            nc.sync.dma_start(wk_sb[:], wkT.rearrange("(kt p) j -> p kt j", p=P))
            kh00[0]()
            nc.sync.dma_start(wq_sb[:], wqT.rearrange("(kt p) j -> p kt j", p=P))
            qh00[0]()
            qh10[0]()
            kh00[1]()
            qh00[1]()
            qh10[1]()

            mtiles = {}

            def mask_dma(qc, c):
                mch = maskp.tile([P, 4, NQ], F16, tag="mch",
                                 name=f"mch{qc}_{c}")
                nc.sync.dma_start(
                    mch[:],
                    maskT[c * 4 * P:(c + 1) * 4 * P,
                          qc * NQ:(qc + 1) * NQ].rearrange(
                        "(kt p) q -> p kt q", p=P),
                )
                mtiles[(qc, c)] = mch

            # proj chain: pair-0 kh blocks first (the stream consumes them
            # within the first head), then the deferred pair-1 blocks, then
            # the v s-tiles.  Each filler prefetches the next unit's chunk.
            chain = ([kh_unit(b, 0) for b in (1, 2, 3)]
                     + [kh_unit(b, 1, act_copy=True) for b in range(4)]
                     + [qh_unit(b, 1, act_copy=True) for b in (0, 1)])
            chunk_dma("k", xkT, 1)
            mask_dma(0, 0)
            chunk_dma("k", xkT, 2)
            chunk_dma("k", xkT, 3)
            mask_dma(0, 1)

            def mk_chain_filler(i):
                def unit():
                    if i == 1:
                        nc.sync.dma_start(
                            wv_sb[:],
                            wvT.rearrange("(kt p) j -> p kt j", p=P))
                        emit_xv_dma()
                    chain[i][1]()
                return unit

            for i in range(len(chain)):
                fillers.append(("p", mk_chain_filler(i)))

            def mk_v_filler(st_):
                vu = mk_v_unit(st_)
                def unit():
                    if st_ % 2 == 0:
                        emit_xv_dma()
                    vu()
                return unit

            for st_ in range(ST):
                fillers.append(("p", mk_v_filler(st_)))

            qh_tail = [qh_unit(2, 0), qh_unit(3, 0),
                       qh_unit(2, 1), qh_unit(3, 1)]

            def mk_qh_tail_filler(i):
                def unit():
                    if i == 0:
                        chunk_dma("q", xqT, 3)
                    elif i == 1:
                        nc.sync.dma_start(
                            wo_sb[:], woT.rearrange("(hp p) e -> p hp e", p=P))
                    qh_tail[i][1]()
                return unit

            # ---- main ACT-paced loop ----
            for qc in range(QC):
                for h in range(HL):
                    hp, hh = divmod(h, 2)
                    p_ab = tuple(
                        pstg.tile([P, 4, NQ], F16, tag="pt",
                                  name=f"pt{qc}_{h}_{i}")
                        for i in range(4))
                    for pr in range(ST // 2):
                        if h == 0 and (qc, pr // 2) not in mtiles:
                            mask_dma(qc, pr // 2)
                        mcur = mtiles[(qc, pr // 2)]
                        for k2 in range(2):
                            ks = pr * 2 + k2
                            st_ = scps.tile([P, NQ], F32, tag="st")
                            for n2 in range(2):
                                nc.tensor.matmul(
                                    st_[:, n2 * 512:(n2 + 1) * 512],
                                    khT[hh * 64:(hh + 1) * 64, hp,
                                        ks * P:(ks + 1) * P],
                                    qhT[hh * 64:(hh + 1) * 64, hp,
                                        qc * NQ + n2 * 512:
                                        qc * NQ + (n2 + 1) * 512],
                                    start=True, stop=True,
                                )
                            ph = p_ab[ks // 4][:, ks % 4, :]
                            nc.scalar.activation(ph, st_[:], Exp, scale=0.125)
                            eng = nc.gpsimd if ks % 8 == 3 else nc.vector
                            eng.tensor_mul(ph, ph, mcur[:, ks % 4, :])
                            pop_filler()
                    for qt in range(QT):
                        fillers.append(("s2", mk_stage2(qc, h, qt, p_ab)))
                    if qc == 0 and h == 0:
                        chunk_dma("q", xqT, 2)
                        for i in range(len(qh_tail)):
                            fillers.append(("p", mk_qh_tail_filler(i)))

            prio = {"p": 0, "s2": 1, "c1": 2, "c2": 3}
            while fillers:
                batch = sorted(fillers, key=lambda kf: prio[kf[0]])
                fillers.clear()
                for _, fn in batch:
                    fn()

    nc.compile()
    return nc


def kernel(q, k, v, mask, Wq, bq, Wk, bk, Wv, bv, Wo, bo):
    global LAST_RESULTS, LAST_NC
    q = np.asarray(q, np.float32)
    k = np.asarray(k, np.float32)
    v = np.asarray(v, np.float32)
    mask = np.asarray(mask)
    Wq = np.asarray(Wq, np.float32)
    Wk = np.asarray(Wk, np.float32)
    Wv = np.asarray(Wv, np.float32)
    Wo = np.asarray(Wo, np.float32)
    bq = np.asarray(bq, np.float32)
    bk = np.asarray(bk, np.float32)
    bv = np.asarray(bv, np.float32)
    bo = np.asarray(bo, np.float32)

    use_bias_qk = bool(np.any(bq) or np.any(bk))
    use_bias_v = bool(np.any(bv))

    nc = _build_program(use_bias_qk, use_bias_v)
    LAST_NC = nc

    xT = {}
    for b in range(B):
        xT[("q", b)] = _f16(q[b].T)
        xT[("k", b)] = _f16(k[b].T)
        xT[("v", b)] = _f16(v[b].T)
        xT[("m", b)] = _f16(mask[b, 0].T.astype(np.float32))

    in_maps = []
    for c in range(8):
        b, g = divmod(c, G)
        js = slice(g * J, (g + 1) * J)
        in_maps.append({
            "xqT": xT[("q", b)],
            "xkT": xT[("k", b)],
            "xvT": xT[("v", b)],
            "maskT": xT[("m", b)],
            "wqT": _f16(Wq[js, :].T),
            "wkT": _f16(Wk[js, :].T),
            "wvT": _f16(Wv[js, :].T),
            "woT": _f16(Wo[:, js].T),
            "bq": np.ascontiguousarray(bq[js]),
            "bk": np.ascontiguousarray(bk[js]),
            "bv": np.ascontiguousarray(bv[js]),
        })

    os.environ["BASS_NEVER_TRACE"] = "1"
    res = bass_utils.run_bass_kernel_spmd(
        nc, in_maps, core_ids=list(range(8)), trace=False,
    )
    LAST_RESULTS = res

    full = np.zeros((B, S, E), np.float32)
    for c in range(8):
        b = c // G
        full[b] += res.results[c]["out"].astype(np.float32)
    full += bo[None, None, :]
    return full
